# revision 40
# baseline (speedup 1.0000x reference)
"""Trainium2 Bass kernel for nn_DifferentiableTortuosity.

Math: 50 iterations of D = min(D, (conv4(D)/4 + 1) * ip) on a (B,512,512)
grid, sampled at start_coords. Information propagates 1 cell/iteration, so
D^50[start] depends only on cells within L1 distance 50 of start — a diamond
of 5101 cells. The grid is rotated 45 degrees: (dr,dc) -> (u,v) =
(dr+dc+50, dr-dc), which maps the diamond to a 101x101 checkerboard box and
the 4-neighbor stencil to the 4 (u+-1, v+-1) diagonal neighbors. Only one
parity of v is occupied per u row, so v packs into 51 columns per batch
window: j = (v+50)/2 on even u rows and — boustrophedon — j = (51-v)/2 on
odd rows (odd rows stored reversed). With that reversal BOTH parities read
their neighbors at the same reversed column pair {50-j, 51-j}, so the whole
4-neighbor + const sum collapses to TWO PSUM-accumulated matmuls with
tridiagonal-over-u stationaries and negative-stride moving slices:
  mm1: T4 (tridiag + const-4 row) @ D[:, rev(51-j)]
  mm2: T  (tridiag)               @ D[:, rev(50-j)]
The j-band shrinks with the dependency cone; band cells outside the exact
cone only ever feed future out-of-cone cells, so the center stays exact.

The elementwise update runs in E-space (E = D / ip4, so
D' = min(D, S4*ip4) <=> E' = min(E, S4), exact since ip4 > 0): the DVE min
consumes the PSUM directly (GPSIMD cannot access PSUM on real HW — BIR
verifier rejects it) and the Pool mult that rebuilds D = E*ip4 for the
next matmul is SBUF-only:
  PE:   S4 = A@D + 4          (2 fp16 matmuls -> fp32 PSUM, accumulated)
  DVE:  E' = min(E, S4)       (fp16 x fp32-PSUM)
  Pool: D' = E' * ip4         (all-SBUF fp16 tensor_tensor mult)

dtypes: everything fp16 (1024 = H+W exact; S4 <= 4100 fits; E = D/ip4 <=
4096). ip4 = 0.25/(p+eps) clipped to 65504 then fp16 — clipped cells have
eff >= 1024 either way, so behavior is identical. The output is read from
E: a center bitwise-equal to E0 means the min never fired there, so the
path length is exactly H+W; otherwise pl = E*ip4 recomputed in fp32 on
host.

Layout per core: 8 batch windows in 51-col slots of one [102, 408] fp16
tile; row 101 is the constant 1.0 row that T4's 4.0 row multiplies to fold
the +4 into mm1. All reads/writes stay inside each 51-col slot (no guard
cols needed). The 8 batches split into 3 groups (2/3/3) with independent
dependency chains: per-iteration wall time is bound by the serial loop
min -> sem -> matmuls -> sem -> min -> sem -> mult (~100ns per cross-engine
hop plus the DVE PSUM-access latency), so the group count trades per-hop
payload on the critical path against the per-op PSUM-access overhead
(CoreSim: G=2 38.5us, G=3 35.2us, G=4 41.5us).
"""
import numpy as np

B_FULL = 64
H = 512
W = 512
NCORES = 8
BPC = B_FULL // NCORES  # 8 batches per core
R = 50
NU = 101          # u rows
NJ = 51           # packed j cols per window
SLOT = 51
WCOLS = SLOT * BPC  # 408
NUM_ITER = 50
EPS = 1e-06
CU, CJ = 50, 25   # center cell in rotated-packed coords

_COMPILED = {}

V4_GROUPS = 3
V4_PS_BUFS = 2
# run the D-rebuild mult on DVE (in-order after the min, saving the
# Pool hop) when the band width is at most this; Pool otherwise
V4_MULT_DVE_W = 0


def _bands(n_iter):
    """Per-iteration packed j band [jlo, jhi] (union over row parities,
    boustrophedon packing)."""
    out = []
    for it in range(1, n_iter + 1):
        itc = min(it, NUM_ITER)  # beyond 50 iters the cone is a point
        jlo = (itc + 1) // 2
        jhi = (101 - itc) // 2
        out.append((jlo, jhi))
    return out


def _build_program(n_iter=NUM_ITER):
    import concourse.bacc as bacc
    import concourse.tile as tile
    from concourse import mybir

    nc = bacc.Bacc("TRN2", target_bir_lowering=False, debug=False,
                   num_devices=NCORES)
    ip_in = nc.declare_dram_parameter("ip4win", [NU, WCOLS], mybir.dt.float16,
                                      isOutput=False)
    d0_in = nc.declare_dram_parameter("d0win", [NU + 1, WCOLS],
                                      mybir.dt.float16, isOutput=False)
    e0_in = nc.declare_dram_parameter("e0win", [NU, WCOLS],
                                      mybir.dt.float16, isOutput=False)
    pl_out = nc.declare_dram_parameter("plens", [1, BPC], mybir.dt.float16,
                                       isOutput=True)

    # stationaries blob: cols 0..100 = T4 (tridiag + 4-row), 101..201 = T
    tt_np = np.zeros((NU + 1, 2 * NU), dtype=np.float16)
    for p in range(NU):
        for k in (p - 1, p + 1):
            if 0 <= k < NU:
                tt_np[k, p] = 1.0
                tt_np[k, NU + p] = 1.0
    tt_np[NU, 0:NU] = 4.0
    tt_dram = nc.inline_tensor(tt_np, "tt")

    G = V4_GROUPS
    gb = BPC // G
    rem = BPC - gb * G
    bounds = []
    b0 = 0
    for gi in range(G):
        b1 = b0 + gb + (1 if gi >= G - rem else 0)  # (2,3,3) beats (3,3,2)
        bounds.append((b0, b1))
        b0 = b1

    with tile.TileContext(nc) as tc:
        with (
            tc.tile_pool(name="state", bufs=1) as state,
            tc.tile_pool(name="ps", bufs=V4_PS_BUFS, space="PSUM") as ps,
        ):
            D = state.tile([NU + 1, WCOLS], mybir.dt.float16)
            E = state.tile([NU, WCOLS], mybir.dt.float16)
            IP4 = state.tile([NU, WCOLS], mybir.dt.float16)
            TT = state.tile([NU + 1, 2 * NU], mybir.dt.float16)
            nc.sync.dma_start(out=D[:], in_=d0_in[:])
            nc.scalar.dma_start(out=TT[:], in_=tt_dram[:])
            nc.gpsimd.dma_start(out=E[:], in_=e0_in[:])
            nc.gpsimd.dma_start(out=IP4[:], in_=ip_in[:])
            Dv = D[:].rearrange("p (b s) -> p b s", s=SLOT)
            Ev = E[:].rearrange("p (b s) -> p b s", s=SLOT)
            IPv = IP4[:].rearrange("p (b s) -> p b s", s=SLOT)

            def rev(view, hi_start, w):
                # cols hi_start, hi_start-1, ..., hi_start-(w-1)
                stop = hi_start - w
                if stop < 0:
                    return view[:, :, hi_start::-1]
                return view[:, :, hi_start:stop:-1]

            for it, (jlo, jhi) in enumerate(_bands(n_iter), start=1):
                w = jhi - jlo + 1
                for gi, (b0, b1) in enumerate(bounds):
                    PS = ps.tile([NU, (b1 - b0) * w], mybir.dt.float32,
                                 tag=f"v{gi}")
                    nc.tensor.matmul(PS[:], TT[:, 0:NU],
                                     rev(Dv[0:NU + 1, b0:b1], 51 - jlo, w),
                                     start=True, stop=False)
                    nc.tensor.matmul(PS[:], TT[0:NU, NU:2 * NU],
                                     rev(Dv[0:NU, b0:b1], 50 - jlo, w),
                                     start=False, stop=True)
                    # E-space min reads the PSUM directly (GPSIMD cannot
                    # touch PSUM on HW); the mult rebuilding D is SBUF-only
                    nc.vector.tensor_tensor(
                        Ev[:, b0:b1, jlo:jlo + w],
                        Ev[:, b0:b1, jlo:jlo + w], PS[:],
                        op=mybir.AluOpType.min)
                    if it == n_iter:
                        continue  # no next matmul: D rebuild unnecessary
                    meng = nc.vector if w <= V4_MULT_DVE_W else nc.gpsimd
                    meng.tensor_tensor(
                        Dv[0:NU, b0:b1, jlo:jlo + w],
                        Ev[:, b0:b1, jlo:jlo + w],
                        IPv[:, b0:b1, jlo:jlo + w],
                        op=mybir.AluOpType.mult)

            nc.sync.dma_start(out=pl_out[:],
                              in_=Ev[CU:CU + 1, :, CJ:CJ + 1])

    nc.compile()
    return nc


# rotated-boustrophedon index maps (module-level, computed once)
_UU, _JJ = np.meshgrid(np.arange(NU), np.arange(NJ), indexing="ij")
_VV = np.where(_UU % 2 == 0, 2 * _JJ - 50, 51 - 2 * _JJ)
_DU = _UU - 50
_DR = (_DU + _VV) // 2
_DC = (_DU - _VV) // 2
_IN_DIAMOND = np.abs(_VV) <= 50


def _prepare_core_inputs(pm, start, goal):
    """pm: (BPC,512,512) f32; start/goal: (BPC,2) int64 (already clipped).
    Returns ip4win (NU, WCOLS) f16, d0win (NU+1, WCOLS) f16,
    e0win (NU, WCOLS) f16 and qc (BPC,) f32 (exact center ip4 values)."""
    ipwin = np.zeros((NU, WCOLS), dtype=np.float16)
    d0win = np.zeros((NU + 1, WCOLS), dtype=np.float16)
    e0win = np.zeros((NU, WCOLS), dtype=np.float16)
    d0win[NU, :] = np.float16(1.0)  # const row for the matmul 4-fold
    big = np.float32(H + W)
    qc = np.zeros(BPC, dtype=np.float32)
    for b in range(BPC):
        sr, sc = int(start[b, 0]), int(start[b, 1])
        r = sr + _DR
        c = sc + _DC
        inmap = (r >= 0) & (r < H) & (c >= 0) & (c < W) & _IN_DIAMOND
        ip4 = np.full((NU, NJ), np.float32(0.25), dtype=np.float32)
        rcl = np.clip(r, 0, H - 1)
        ccl = np.clip(c, 0, W - 1)
        vals = np.float32(0.25) / (pm[b][rcl, ccl] + np.float32(EPS))
        ip4 = np.where(inmap, vals, ip4)
        d0 = np.where(inmap, big, np.float32(0.0)).astype(np.float32)
        gdr, gdc = int(goal[b, 0]) - sr, int(goal[b, 1]) - sc
        gu, gv = gdr + gdc + 50, gdr - gdc
        if 0 <= gu <= 100 and abs(gv) <= 50:
            gj = (gv + 50) // 2 if gu % 2 == 0 else (51 - gv) // 2
            if 0 <= gj < NJ:
                d0[gu, gj] = 0.0
        cb = SLOT * b
        ip16 = np.minimum(ip4, 65504.0).astype(np.float16)
        d16 = d0.astype(np.float16)
        ipwin[:, cb:cb + NJ] = ip16
        d0win[0:NU, cb:cb + NJ] = d16
        e0win[:, cb:cb + NJ] = (d16.astype(np.float32)
                                / ip16.astype(np.float32)).astype(np.float16)
        qc[b] = ip16[CU, CJ]
    return ipwin, d0win, e0win, qc


def kernel(probability_map, start_coords, goal_coords, _n_iter=NUM_ITER):
    from concourse.bass_utils import run_bass_kernel_spmd

    pm = np.asarray(probability_map, dtype=np.float32)
    sc_all = np.asarray(start_coords)
    gc_all = np.asarray(goal_coords)
    B = pm.shape[0]
    assert pm.shape == (B_FULL, 1, H, W) and B == B_FULL

    sr = np.clip(sc_all[:, 0], 0, H - 1).astype(np.int64)
    sc = np.clip(sc_all[:, 1], 0, W - 1).astype(np.int64)
    gr = np.clip(gc_all[:, 0], 0, H - 1).astype(np.int64)
    gc = np.clip(gc_all[:, 1], 0, W - 1).astype(np.int64)
    start = np.stack([sr, sc], axis=1)
    goal = np.stack([gr, gc], axis=1)

    if _n_iter not in _COMPILED:
        _COMPILED[_n_iter] = _build_program(_n_iter)
    nc = _COMPILED[_n_iter]

    in_maps = []
    e0c_all, qc_all = [], []
    for c in range(NCORES):
        lo = c * BPC
        ipwin, d0win, e0win, qc = _prepare_core_inputs(
            pm[lo:lo + BPC, 0], start[lo:lo + BPC], goal[lo:lo + BPC])
        in_maps.append({"ip4win": ipwin, "d0win": d0win, "e0win": e0win})
        e0c_all.append(e0win[CU, CJ::SLOT].copy())
        qc_all.append(qc)

    res = run_bass_kernel_spmd(nc, in_maps, list(range(NCORES)))
    # Center value in E-space: bitwise-equal to E0 means the min never fired
    # there, so the true path length is exactly D0 = H+W; otherwise pl = E*q.
    pls = []
    for c in range(NCORES):
        ec = np.asarray(res.results[c]["plens"]).reshape(BPC)
        untouched = ec == e0c_all[c]
        pl = ec.astype(np.float32) * qc_all[c]
        pls.append(np.where(untouched, np.float32(H + W), pl))
    path_lengths = np.concatenate(pls).astype(np.float32)

    diff = (gc_all - sc_all).astype(np.float32)
    euclid = np.sqrt((diff * diff).sum(axis=1, dtype=np.float32))
    euclid = np.maximum(euclid, np.float32(1.0))
    tortuosity = (path_lengths / euclid).astype(np.float32)
    is_valid = path_lengths < np.float32(H + W)
    return tortuosity, is_valid
